# revision 6
# baseline (speedup 1.0000x reference)
"""Trainium2 Bass kernel for nn_Attention_57423712748130.

Computation (per batch b):
  X4 = x[b] viewed (C=256, N=4096)   [raw reshape]
  Q4 = silu(BN(q_w @ X4))            (256, 4096)
  KV4 = silu(BN(kv_w @ Y4))          (128, 4096)
  q[n,h,d]  = Q4[n1, n0*256+h*64+d]      n = n1*16+n0
  k[m,h,d]  = KV4[m1, m0*512 + h*64+d]   m = m1*8+m0
  v[m,h,d]  = KV4[m1, m0*512+256+h*64+d]
  att = softmax(q k^T / 8); o = att v
  out rows [h*1024,(h+1)*1024) = O_h @ proj_w.T + proj_b
    where O_h[n2, n3*64+d] = o[4*n2+n3, d]

Sharding: 8 cores = (batch b in 0..3) x (head-pair hp in 0..1); each core
computes heads {2hp, 2hp+1} of batch b = rows [hp*2048,(hp+1)*2048) of out[b].

On-core strategy (v2 — ScalarE(exp)-bound design, ~all else overlapped):
 - all bulk inputs/weights arrive as bf16 (host casts); conv outputs are
   computed directly in transposed layout so q^T/k^T need no transposes
 - conv bias is folded into the conv's PSUM accumulation as a leading K=1
   matmul (ones (x) bias_row); silu computed as z*(1+tanh(z/2)) = 2*silu(z)
   (tanh shares the ACT table set with exp); the 2x is folded into the exp
   scale (1/32) and the 2.0 ones-column of V
 - scoresT[m,n] = k^T.T @ q^T in PSUM (two heads on PE row groups 0-63 /
   64-127, running concurrently), one [128,1024] exp per 2-bank psum
 - att@v contracts over m with an extra 2.0-column on V producing softmax
   denominators as psum row 64; 1/denom via reciprocal_approx_fast straight
   off the PSUM row, broadcast across partitions with gpsimd
   partition_broadcast, and the normalize multiply is folded into the
   PSUM-evacuation copy (tensor_mul) — no DRAM bounce, no grid DMAs
 - normalization + projection run per q0-quarter, pipelined behind the
   attention of later quarters; O_h columns kept q0-major so every PSUM
   evacuation is contiguous; the final output DMA un-permutes rows
 - q convs are emitted software-pipelined inside the attention loop so the
   ScalarE FIFO (the bottleneck engine: 64 exp + 12 tanh ops) never blocks;
   DMA issues are spread over sync/gpsimd/vector/tensor, none on ScalarE
"""

import ml_dtypes
import numpy as np

B = 4
N_TOK = 4096
C = 256
BN_EPS = 1e-5

_CACHE = {}


def _build():
    import concourse.bacc as bacc
    import concourse.bass as bass
    import concourse.tile as tile
    from concourse import mybir

    f32 = mybir.dt.float32
    bf16 = mybir.dt.bfloat16
    adt = bf16
    AF = mybir.ActivationFunctionType

    nc = bacc.Bacc("TRN2", target_bir_lowering=False, debug=False, num_devices=8)

    xq = nc.dram_tensor("xq", [256, 2048], bf16, kind="ExternalInput")
    yk = nc.dram_tensor("yk", [256, 1024], bf16, kind="ExternalInput")
    yv = nc.dram_tensor("yv", [256, 1024], bf16, kind="ExternalInput")
    wq = nc.dram_tensor("wq", [256, 256], bf16, kind="ExternalInput")
    bq = nc.dram_tensor("bq", [1, 512], bf16, kind="ExternalInput")
    wkv = nc.dram_tensor("wkv", [256, 128], bf16, kind="ExternalInput")
    bkv4 = nc.dram_tensor("bkv4", [1, 512], bf16, kind="ExternalInput")
    bkvr = nc.dram_tensor("bkvr", [1, 128], bf16, kind="ExternalInput")
    wp = nc.dram_tensor("wp", [256, 256], bf16, kind="ExternalInput")
    bp = nc.dram_tensor("bp", [1, 512], f32, kind="ExternalInput")
    out = nc.dram_tensor("out", [2048, 256], f32, kind="ExternalOutput")

    with tile.TileContext(nc) as tc:
        with (
            tc.tile_pool(name="const", bufs=1) as cp,
            tc.tile_pool(name="actt", bufs=3) as actt,
            tc.tile_pool(name="attp", bufs=16) as attp,
            tc.tile_pool(name="outp", bufs=3) as outp,
            tc.tile_pool(name="nrm", bufs=3) as nrm,
            tc.tile_pool(name="gp", bufs=3) as gp,
            tc.tile_pool(name="psc", bufs=1, space="PSUM") as psc,
            tc.tile_pool(name="pss", bufs=2, space="PSUM") as pss,
            tc.tile_pool(name="pso", bufs=3, space="PSUM") as pso,
        ):
            # ---- load weights / inputs (DMA issues on gpsimd+sync only) ----
            # [256, W] DRAM tensors land as [128, 2, W] SBUF tiles (row half
            # c0 side by side) so one DMA covers both halves of a col piece.
            def folded(t_dram, w, tag, eng, piece, dt=bf16):
                t = cp.tile([128, 2, w], dt, tag=tag, name=tag)
                for p0 in range(0, w, piece):
                    eng.dma_start(
                        t[:, :, p0 : p0 + piece],
                        bass.AP(
                            tensor=t_dram, offset=p0,
                            ap=[[w, 128], [128 * w, 2], [1, piece]]))
                return t

            def load_row(t_dram, shape, tag, eng, dt=bf16):
                t = cp.tile(shape, dt, tag=tag, name=tag)
                eng.dma_start(t[:], t_dram.ap())
                return t

            # small operands first so the kv conv can start ~immediately
            bkv4_sb = load_row(bkv4, [1, 512], "bkv4", nc.gpsimd)
            wkvc = folded(wkv, 128, "wkv", nc.gpsimd, 128)
            ykc = folded(yk, 1024, "yk", nc.gpsimd, 512)
            bq_sb = load_row(bq, [1, 512], "bq", nc.sync)
            wqc = folded(wq, 256, "wq", nc.sync, 256)
            xqc = folded(xq, 2048, "xq", nc.sync, 512)
            bkvr_sb = load_row(bkvr, [1, 128], "bkvr", nc.gpsimd)
            yvc = folded(yv, 1024, "yv", nc.gpsimd, 512)
            wpc = folded(wp, 256, "wp", nc.sync, 256)
            bp_bc = cp.tile([128, 512], f32, tag="bp_bc", name="bp_bc")
            nc.gpsimd.dma_start(bp_bc[:], bp.ap().partition_broadcast(128))
            wkv_sb = [wkvc[:, i, :] for i in range(2)]
            yk_sb = [ykc[:, i, :] for i in range(2)]
            wq_sb = [wqc[:, i, :] for i in range(2)]
            xq_sb = [xqc[:, i, :] for i in range(2)]
            yv_sb = [yvc[:, i, :] for i in range(2)]
            wp_sb = [wpc[:, i, :] for i in range(2)]

            ones_row = cp.tile([1, 512], bf16, tag="ones", name="ones")
            nc.vector.memset(ones_row[:], 1.0)

            # conv epilogue: psum z already includes bias (K=1 bias matmul).
            # t = tanh(z/2); u = z*t; dst = z + u = z*(1+tanh(z/2)) = 2silu(z)
            def silu_epi(ps, dst_ap, tag, rr=None):
                t = actt.tile([128, 512], f32, tag="silu_t", name=f"t_{tag}")
                u = actt.tile([128, 512], f32, tag="silu_u", name=f"u_{tag}")
                nc.scalar.activation(t[:], ps, AF.Tanh, scale=0.5)
                nc.vector.tensor_mul(u[:], ps, t[:])
                psv, uv = ps, u[:]
                if rr is not None:
                    psv = psv.rearrange(rr, a=4, h=2)
                    uv = uv.rearrange(rr, a=4, h=2)
                nc.vector.tensor_add(dst_ap, psv, uv)

            # ---- kv conv (k part): kT[pp, m0, m1], pp = hl*64+d ----
            kT = cp.tile([128, 8, 128], adt, tag="kT")
            for mt in range(2):  # m0 quads
                ps = psc.tile([128, 512], f32, tag="cnv", name=f"psk{mt}")
                nc.tensor.matmul(
                    ps[:], lhsT=ones_row[:, 0:128], rhs=bkv4_sb[:],
                    start=True, stop=False)
                for mi in range(4):
                    m0 = 4 * mt + mi
                    for c0 in range(2):
                        nc.tensor.matmul(
                            ps[:, mi * 128 : (mi + 1) * 128],
                            lhsT=yk_sb[c0][:, m0 * 128 : (m0 + 1) * 128],
                            rhs=wkv_sb[c0][:],
                            start=False, stop=(mi == 3 and c0 == 1))
                silu_epi(
                    ps[:],
                    kT[:, 4 * mt : 4 * mt + 4, :].rearrange("p a b -> p (a b)"),
                    f"k{mt}")

            # ---- kv conv (v part): vext[m1, m0, hl, 0:64]=2v, [...,64]=2 ----
            vext = cp.tile([128, 8, 2, 65], adt, tag="vext")
            nc.vector.memset(vext[:], 2.0)
            for jv in range(2):
                ps = psc.tile([128, 512], f32, tag="cnv", name=f"psv{jv}")
                nc.tensor.matmul(
                    ps[:], lhsT=bkvr_sb[:], rhs=ones_row[:],
                    start=True, stop=False)
                for c0 in range(2):
                    nc.tensor.matmul(
                        ps[:], lhsT=wkv_sb[c0][:],
                        rhs=yv_sb[c0][:, jv * 512 : (jv + 1) * 512],
                        start=False, stop=(c0 == 1))
                silu_epi(
                    ps[:], vext[:, jv * 4 : (jv + 1) * 4, :, 0:64],
                    f"v{jv}", rr="p (a h d) -> p a h d")

            # ---- attention state (both heads) ----
            qT = cp.tile([128, 16, 256], adt, tag="qT")
            outun = [
                [cp.tile([128, 1024], adt, tag=f"outun{hl}_{i}",
                         name=f"outun{hl}_{i}") for i in range(2)]
                for hl in range(2)
            ]

            # q conv for one n0-pair
            def q_conv(t2):
                ps = psc.tile([128, 512], f32, tag="cnv", name=f"psq{t2}")
                nc.tensor.matmul(
                    ps[:], lhsT=ones_row[:, 0:128], rhs=bq_sb[:],
                    start=True, stop=False)
                for nn in range(2):
                    n0 = 2 * t2 + nn
                    for c0 in range(2):
                        nc.tensor.matmul(
                            ps[:, nn * 256 : (nn + 1) * 256],
                            lhsT=xq_sb[c0][:, n0 * 128 : (n0 + 1) * 128],
                            rhs=wq_sb[c0][:],
                            start=False, stop=(nn == 1 and c0 == 1))
                silu_epi(
                    ps[:],
                    qT[:, 2 * t2 : 2 * t2 + 2, :].rearrange("p a b -> p (a b)"),
                    f"q{t2}")

            q_conv(0)
            q_conv(1)

            # t2-outer attention; scores for hl=0/1 sit on array row-groups
            # 0-63 / 64-127, emitted adjacently for PE row-group overlap.
            for t2 in range(8):  # n0 pair (n0 = 2*t2 + nn)
                att = {0: [], 1: []}
                for j in range(4):  # m0 = 2j + mi
                    scps = {}
                    for hl in range(2):
                        scps[hl] = pss.tile([128, 1024], f32, tag="scp",
                                            name=f"scp{hl}_{t2}_{j}")
                    for mi in range(2):
                        m0 = 2 * j + mi
                        for hl in range(2):
                            r0, r1 = hl * 64, (hl + 1) * 64
                            nc.tensor.matmul(
                                scps[hl][:, mi * 512 : (mi + 1) * 512],
                                lhsT=kT[r0:r1, m0, :],
                                rhs=qT[r0:r1, 2 * t2 : 2 * t2 + 2, :],
                                start=True, stop=True)
                    for hl in range(2):
                        a = attp.tile([128, 1024], adt, tag="att",
                                      name=f"att{hl}_{t2}_{j}")
                        # scoresT = 4*q.k ; want exp(q.k/8) -> scale 1/32
                        nc.scalar.activation(
                            a[:], scps[hl][:], AF.Exp, scale=0.03125)
                        att[hl].append(a)

                # software-pipelined q conv (PE work fills exp-wait)
                if t2 + 2 < 8:
                    q_conv(t2 + 2)

                opss = {}
                for hl in range(2):
                    ops = pso.tile([65, 512], f32, tag="ops",
                                   name=f"ops{hl}_{t2}")
                    opss[hl] = ops
                    for m0 in range(8):
                        nc.tensor.matmul(
                            ops[:], lhsT=vext[:, m0, hl, :],
                            rhs=att[hl][m0 // 2][
                                :, (m0 % 2) * 512 : (m0 % 2 + 1) * 512],
                            start=(m0 == 0), stop=(m0 == 7))

                # normalize: 1/denom straight off psum row 64, partition-
                # broadcast, multiply folded into the PSUM evacuation
                for hl in range(2):
                    ops = opss[hl]
                    # recip_approx_fast is wrong straight off PSUM (HW bit-
                    # trick breaks on the PSUM read path) — copy to SBUF first
                    drow = nrm.tile([1, 512], f32, tag="drow",
                                    name=f"drow{hl}_{t2}")
                    nc.vector.tensor_copy(drow[:], ops[64:65, :])
                    rrow = nrm.tile([1, 512], f32, tag="rrow",
                                    name=f"rrow{hl}_{t2}")
                    nc.vector.reciprocal_approx_fast(
                        out=rrow[:], in_=drow[:])
                    g = gp.tile([64, 512], f32, tag="g", name=f"g{hl}_{t2}")
                    nc.gpsimd.partition_broadcast(g[:], rrow[:], channels=64)
                    c0 = t2 % 2
                    q0 = t2 // 2
                    for nn in range(2):
                        # o * (1/denom) -> outun[c][band, q0-major cols]
                        dst = outun[hl][c0][
                            nn * 64 : nn * 64 + 64,
                            q0 * 256 : (q0 + 1) * 256]
                        nc.vector.tensor_mul(
                            dst,
                            ops[0:64, nn * 256 : (nn + 1) * 256],
                            g[0:64, nn * 256 : (nn + 1) * 256])

                if t2 % 2 == 0:
                    continue
                # ---- quarter q0 = t2//2 complete: projection ----
                q0 = t2 // 2
                for hl in range(2):
                    # proj fc in {2q0, 2q0+1}; rows hl*1024+half*512+q0+4r
                    ps2 = psc.tile([128, 512], f32, tag="cnv",
                                   name=f"psproj{hl}_{q0}")
                    for half in range(2):
                        fc = 2 * q0 + half
                        for c0 in range(2):
                            nc.tensor.matmul(
                                ps2[:, half * 256 : (half + 1) * 256],
                                lhsT=outun[hl][c0][
                                    :, fc * 128 : (fc + 1) * 128],
                                rhs=wp_sb[c0][:],
                                start=(c0 == 0), stop=(c0 == 1))
                    osb = outp.tile([128, 512], f32, tag="osb",
                                    name=f"osb{hl}_{q0}")
                    nc.vector.tensor_add(osb[:], ps2[:], bp_bc[:])
                    dstap = bass.AP(
                        tensor=out,
                        offset=(hl * 1024 + q0) * 256,
                        ap=[[4 * 256, 128], [512 * 256, 2], [1, 256]])
                    nc.sync.dma_start(
                        dstap,
                        osb[:].rearrange("p (h c) -> p h c", h=2))

    nc.compile()
    return nc


def _prep_inputs(x, y, q_w, q_gamma, q_beta, q_mean, q_var,
                 kv_w, kv_gamma, kv_beta, kv_mean, kv_var, proj_w, proj_b):
    f = np.float32
    bf = ml_dtypes.bfloat16
    x = np.ascontiguousarray(np.asarray(x, f))
    y = np.ascontiguousarray(np.asarray(y, f))

    gq = np.asarray(q_gamma, f) / np.sqrt(np.asarray(q_var, f) + BN_EPS)
    bq_full = np.asarray(q_beta, f) - np.asarray(q_mean, f) * gq
    wq_host = np.ascontiguousarray((np.asarray(q_w, f) * gq[:, None]).T).astype(bf)

    gkv = np.asarray(kv_gamma, f) / np.sqrt(np.asarray(kv_var, f) + BN_EPS)
    bkv_full = np.asarray(kv_beta, f) - np.asarray(kv_mean, f) * gkv
    wkv_host = np.ascontiguousarray((np.asarray(kv_w, f) * gkv[:, None]).T).astype(bf)

    wp_host = np.ascontiguousarray(np.asarray(proj_w, f).T).astype(bf)
    bp_host = np.asarray(proj_b, f)

    bq2 = np.tile(bq_full[None, :], (1, 2)).astype(bf)
    bkv4_h = np.tile(bkv_full[None, :], (1, 4)).astype(bf)
    bkvr_h = bkv_full[None, :].astype(bf)
    bp2 = np.tile(bp_host[None, :], (1, 2)).astype(f)

    in_maps = []
    for core in range(8):
        b, hp = core // 2, core % 2
        X4 = x[b].reshape(C, N_TOK)
        Y4 = y[b].reshape(C, N_TOK)
        xqa = np.ascontiguousarray(
            X4.reshape(C, 16, 256)[:, :, hp * 128 : (hp + 1) * 128]
        ).reshape(C, 2048).astype(bf)
        Y8 = Y4.reshape(C, 8, 512)
        yka = np.ascontiguousarray(
            Y8[:, :, hp * 128 : (hp + 1) * 128]).reshape(C, 1024).astype(bf)
        yva = np.ascontiguousarray(
            Y8[:, :, 256 + hp * 128 : 256 + (hp + 1) * 128]
        ).reshape(C, 1024).astype(bf)
        in_maps.append({
            "xq": xqa, "yk": yka, "yv": yva,
            "wq": wq_host, "bq": bq2,
            "wkv": wkv_host, "bkv4": bkv4_h, "bkvr": bkvr_h,
            "wp": wp_host, "bp": bp2,
        })
    return in_maps


def _get_nc():
    if "nc" not in _CACHE:
        _CACHE["nc"] = _build()
    return _CACHE["nc"]


def kernel(x, y, H=64, W=64, q_w=None, q_gamma=None, q_beta=None, q_mean=None,
           q_var=None, kv_w=None, kv_gamma=None, kv_beta=None, kv_mean=None,
           kv_var=None, proj_w=None, proj_b=None, _trace=False):
    from concourse.bass_utils import run_bass_kernel_spmd

    nc = _get_nc()
    in_maps = _prep_inputs(x, y, q_w, q_gamma, q_beta, q_mean, q_var,
                           kv_w, kv_gamma, kv_beta, kv_mean, kv_var,
                           proj_w, proj_b)
    kw = {}
    if _trace:
        kw = {"trace": True, "trace_cores": list(range(8))}
    res = run_bass_kernel_spmd(nc, in_maps, list(range(8)), **kw)
    outa = np.empty((B, N_TOK, C), np.float32)
    for core in range(8):
        b, hp = core // 2, core % 2
        outa[b, hp * 2048 : (hp + 1) * 2048, :] = res.results[core]["out"]
    if _trace:
        return outa, res
    return outa


# revision 9
# speedup vs baseline: 1.3505x; 1.3505x over previous
"""Trainium2 Bass kernel for nn_Attention_57423712748130.

Computation (per batch b):
  X4 = x[b] viewed (C=256, N=4096)   [raw reshape]
  Q4 = silu(BN(q_w @ X4))            (256, 4096)
  KV4 = silu(BN(kv_w @ Y4))          (128, 4096)
  q[n,h,d]  = Q4[n1, n0*256+h*64+d]      n = n1*16+n0
  k[m,h,d]  = KV4[m1, m0*512 + h*64+d]   m = m1*8+m0
  v[m,h,d]  = KV4[m1, m0*512+256+h*64+d]
  att = softmax(q k^T / 8); o = att v
  out rows [h*1024,(h+1)*1024) = O_h @ proj_w.T + proj_b
    where O_h[n2, n3*64+d] = o[4*n2+n3, d]

Sharding: 8 cores = (batch b in 0..3) x (head-pair hp in 0..1); each core
computes heads {2hp, 2hp+1} of batch b = rows [hp*2048,(hp+1)*2048) of out[b].

On-core strategy (v2 — ScalarE(exp)-bound design, ~all else overlapped):
 - all bulk inputs/weights arrive as bf16 (host casts); conv outputs are
   computed directly in transposed layout so q^T/k^T need no transposes
 - conv bias is folded into the conv's PSUM accumulation as a leading K=1
   matmul (ones (x) bias_row); silu computed as z*(1+tanh(z/2)) = 2*silu(z)
   (tanh shares the ACT table set with exp); the 2x is folded into the exp
   scale (1/32) and the 2.0 ones-column of V
 - scoresT[m,n] = k^T.T @ q^T in PSUM (two heads on PE row groups 0-63 /
   64-127, running concurrently), one [128,1024] exp per 2-bank psum
 - att@v contracts over m with an extra 2.0-column on V producing softmax
   denominators as psum row 64; 1/denom via reciprocal_approx_fast straight
   off the PSUM row, broadcast across partitions with gpsimd
   partition_broadcast, and the normalize multiply is folded into the
   PSUM-evacuation copy (tensor_mul) — no DRAM bounce, no grid DMAs
 - normalization + projection run per q0-quarter, pipelined behind the
   attention of later quarters; O_h columns kept q0-major so every PSUM
   evacuation is contiguous; the final output DMA un-permutes rows
 - q convs are emitted software-pipelined inside the attention loop so the
   ScalarE FIFO (the bottleneck engine: 64 exp + 12 tanh ops) never blocks;
   DMA issues are spread over sync/gpsimd/vector/tensor, none on ScalarE
"""

import ml_dtypes
import numpy as np

B = 4
N_TOK = 4096
C = 256
BN_EPS = 1e-5

_CACHE = {}


def _build():
    import concourse.bacc as bacc
    import concourse.bass as bass
    import concourse.tile as tile
    from concourse import mybir

    f32 = mybir.dt.float32
    bf16 = mybir.dt.bfloat16
    adt = bf16
    AF = mybir.ActivationFunctionType

    nc = bacc.Bacc("TRN2", target_bir_lowering=False, debug=False, num_devices=8)

    xq = nc.dram_tensor("xq", [256, 2048], bf16, kind="ExternalInput")
    yk = nc.dram_tensor("yk", [256, 1024], bf16, kind="ExternalInput")
    yv = nc.dram_tensor("yv", [256, 1024], bf16, kind="ExternalInput")
    wq = nc.dram_tensor("wq", [256, 256], bf16, kind="ExternalInput")
    bq = nc.dram_tensor("bq", [1, 512], bf16, kind="ExternalInput")
    wkv = nc.dram_tensor("wkv", [256, 128], bf16, kind="ExternalInput")
    bkv4 = nc.dram_tensor("bkv4", [1, 512], bf16, kind="ExternalInput")
    bkvr = nc.dram_tensor("bkvr", [1, 128], bf16, kind="ExternalInput")
    wp = nc.dram_tensor("wp", [256, 256], bf16, kind="ExternalInput")
    bp = nc.dram_tensor("bp", [1, 512], f32, kind="ExternalInput")
    out = nc.dram_tensor("out", [2048, 256], f32, kind="ExternalOutput")

    with tile.TileContext(nc) as tc:
        with (
            tc.tile_pool(name="const", bufs=1) as cp,
            tc.tile_pool(name="actt", bufs=3) as actt,
            tc.tile_pool(name="attp", bufs=16) as attp,
            tc.tile_pool(name="outp", bufs=3) as outp,
            tc.tile_pool(name="nrm", bufs=3) as nrm,
            tc.tile_pool(name="gp", bufs=3) as gp,
            tc.tile_pool(name="psc", bufs=4, space="PSUM") as psc,
            tc.tile_pool(name="pss", bufs=2, space="PSUM") as pss,
        ):
            # ---- load weights / inputs (DMA issues on gpsimd+sync only) ----
            # [256, W] DRAM tensors land as [128, 2, W] SBUF tiles (row half
            # c0 side by side) so one DMA covers both halves of a col piece.
            def folded(t_dram, w, tag, eng, piece, dt=bf16):
                t = cp.tile([128, 2, w], dt, tag=tag, name=tag)
                for p0 in range(0, w, piece):
                    eng.dma_start(
                        t[:, :, p0 : p0 + piece],
                        bass.AP(
                            tensor=t_dram, offset=p0,
                            ap=[[w, 128], [128 * w, 2], [1, piece]]))
                return t

            def load_row(t_dram, shape, tag, eng, dt=bf16):
                t = cp.tile(shape, dt, tag=tag, name=tag)
                eng.dma_start(t[:], t_dram.ap())
                return t

            # small operands first so the kv conv can start ~immediately
            bkv4_sb = load_row(bkv4, [1, 512], "bkv4", nc.gpsimd)
            wkvc = folded(wkv, 128, "wkv", nc.gpsimd, 128)
            ykc = folded(yk, 1024, "yk", nc.gpsimd, 512)
            bq_sb = load_row(bq, [1, 512], "bq", nc.sync)
            wqc = folded(wq, 256, "wq", nc.sync, 256)
            xqc = folded(xq, 2048, "xq", nc.sync, 512)
            bkvr_sb = load_row(bkvr, [1, 128], "bkvr", nc.gpsimd)
            yvc = folded(yv, 1024, "yv", nc.gpsimd, 512)
            wpc = folded(wp, 256, "wp", nc.sync, 256)
            bp_bc = cp.tile([128, 512], f32, tag="bp_bc", name="bp_bc")
            nc.gpsimd.dma_start(bp_bc[:], bp.ap().partition_broadcast(128))
            wkv_sb = [wkvc[:, i, :] for i in range(2)]
            yk_sb = [ykc[:, i, :] for i in range(2)]
            wq_sb = [wqc[:, i, :] for i in range(2)]
            xq_sb = [xqc[:, i, :] for i in range(2)]
            yv_sb = [yvc[:, i, :] for i in range(2)]
            wp_sb = [wpc[:, i, :] for i in range(2)]

            ones_row = cp.tile([1, 512], bf16, tag="ones", name="ones")
            nc.vector.memset(ones_row[:], 1.0)

            # ---- PE warmup burst: ~4.3us of back-to-back matmuls flips the
            # HAM clock gate to 8/8 (2.4 GHz) before real work; overlaps the
            # input-DMA dead time at kernel start.
            wps = psc.tile([128, 512], f32, tag="cnv", name="warm")
            for wi in range(10):
                nc.tensor.matmul(
                    wps[:], lhsT=ones_row[:, 0:128], rhs=ones_row[:],
                    start=(wi == 0), stop=(wi == 9))

            # conv epilogue: psum z already includes bias (K=1 bias matmul).
            # t = tanh(z/2); u = z*t; dst = z + u = z*(1+tanh(z/2)) = 2silu(z)
            def silu_epi(ps, dst_ap, tag, rr=None):
                t = actt.tile([128, 512], f32, tag="silu_t", name=f"t_{tag}")
                u = actt.tile([128, 512], f32, tag="silu_u", name=f"u_{tag}")
                nc.scalar.activation(t[:], ps, AF.Tanh, scale=0.5)
                nc.vector.tensor_mul(u[:], ps, t[:])
                psv, uv = ps, u[:]
                if rr is not None:
                    psv = psv.rearrange(rr, a=4, h=2)
                    uv = uv.rearrange(rr, a=4, h=2)
                nc.vector.tensor_add(dst_ap, psv, uv)

            # ---- kv conv (k part): kT[pp, m0, m1], pp = hl*64+d ----
            kT = cp.tile([128, 8, 128], adt, tag="kT")
            for mt in range(2):  # m0 quads
                ps = psc.tile([128, 512], f32, tag="cnv", name=f"psk{mt}")
                nc.tensor.matmul(
                    ps[:], lhsT=ones_row[:, 0:128], rhs=bkv4_sb[:],
                    start=True, stop=False)
                for mi in range(4):
                    m0 = 4 * mt + mi
                    for c0 in range(2):
                        nc.tensor.matmul(
                            ps[:, mi * 128 : (mi + 1) * 128],
                            lhsT=yk_sb[c0][:, m0 * 128 : (m0 + 1) * 128],
                            rhs=wkv_sb[c0][:],
                            start=False, stop=(mi == 3 and c0 == 1))
                silu_epi(
                    ps[:],
                    kT[:, 4 * mt : 4 * mt + 4, :].rearrange("p a b -> p (a b)"),
                    f"k{mt}")

            # ---- kv conv (v part): vext[m1, m0, hl, 0:64]=2v, [...,64]=2 ----
            vext = cp.tile([128, 8, 2, 65], adt, tag="vext")
            nc.vector.memset(vext[:], 2.0)
            for jv in range(2):
                ps = psc.tile([128, 512], f32, tag="cnv", name=f"psv{jv}")
                nc.tensor.matmul(
                    ps[:], lhsT=bkvr_sb[:], rhs=ones_row[:],
                    start=True, stop=False)
                for c0 in range(2):
                    nc.tensor.matmul(
                        ps[:], lhsT=wkv_sb[c0][:],
                        rhs=yv_sb[c0][:, jv * 512 : (jv + 1) * 512],
                        start=False, stop=(c0 == 1))
                silu_epi(
                    ps[:], vext[:, jv * 4 : (jv + 1) * 4, :, 0:64],
                    f"v{jv}", rr="p (a h d) -> p a h d")

            # ---- attention state (both heads) ----
            qT = cp.tile([128, 16, 256], adt, tag="qT")
            outun = [
                [cp.tile([128, 1024], adt, tag=f"outun{hl}_{i}",
                         name=f"outun{hl}_{i}") for i in range(2)]
                for hl in range(2)
            ]

            # q conv for one n0-pair
            def q_conv(t2):
                ps = psc.tile([128, 512], f32, tag="cnv", name=f"psq{t2}")
                nc.tensor.matmul(
                    ps[:], lhsT=ones_row[:, 0:128], rhs=bq_sb[:],
                    start=True, stop=False)
                for nn in range(2):
                    n0 = 2 * t2 + nn
                    for c0 in range(2):
                        nc.tensor.matmul(
                            ps[:, nn * 256 : (nn + 1) * 256],
                            lhsT=xq_sb[c0][:, n0 * 128 : (n0 + 1) * 128],
                            rhs=wq_sb[c0][:],
                            start=False, stop=(nn == 1 and c0 == 1))
                silu_epi(
                    ps[:],
                    qT[:, 2 * t2 : 2 * t2 + 2, :].rearrange("p a b -> p (a b)"),
                    f"q{t2}")

            q_conv(0)
            q_conv(1)

            # t2-outer attention; scores for hl=0/1 sit on array row-groups
            # 0-63 / 64-127, emitted adjacently for PE row-group overlap.
            for t2 in range(8):  # n0 pair (n0 = 2*t2 + nn)
                att = {0: [], 1: []}
                for j in range(4):  # m0 = 2j + mi
                    scps = {}
                    for hl in range(2):
                        scps[hl] = pss.tile([128, 1024], f32, tag="scp",
                                            name=f"scp{hl}_{t2}_{j}")
                    for mi in range(2):
                        m0 = 2 * j + mi
                        for hl in range(2):
                            r0, r1 = hl * 64, (hl + 1) * 64
                            nc.tensor.matmul(
                                scps[hl][:, mi * 512 : (mi + 1) * 512],
                                lhsT=kT[r0:r1, m0, :],
                                rhs=qT[r0:r1, 2 * t2 : 2 * t2 + 2, :],
                                start=True, stop=True)
                    for hl in range(2):
                        a = attp.tile([128, 1024], adt, tag="att",
                                      name=f"att{hl}_{t2}_{j}")
                        # scoresT = 4*q.k ; want exp(q.k/8) -> scale 1/32
                        nc.scalar.activation(
                            a[:], scps[hl][:], AF.Exp, scale=0.03125)
                        att[hl].append(a)

                # software-pipelined q conv (PE work fills exp-wait)
                if t2 + 2 < 8:
                    q_conv(t2 + 2)

                opss = {}
                for hl in range(2):
                    ops = psc.tile([128, 512], f32, tag="cnv",
                                   name=f"ops{hl}_{t2}")
                    opss[hl] = ops
                    for m0 in range(8):
                        nc.tensor.matmul(
                            ops[0:65, :], lhsT=vext[:, m0, hl, :],
                            rhs=att[hl][m0 // 2][
                                :, (m0 % 2) * 512 : (m0 % 2 + 1) * 512],
                            start=(m0 == 0), stop=(m0 == 7))

                # normalize: 1/denom straight off psum row 64, partition-
                # broadcast, multiply folded into the PSUM evacuation
                for hl in range(2):
                    ops = opss[hl]
                    # recip_approx_fast is wrong straight off PSUM (HW bit-
                    # trick breaks on the PSUM read path) — copy to SBUF first
                    drow = nrm.tile([1, 512], f32, tag="drow",
                                    name=f"drow{hl}_{t2}")
                    nc.vector.tensor_copy(drow[:], ops[64:65, :])
                    rrow = nrm.tile([1, 512], f32, tag="rrow",
                                    name=f"rrow{hl}_{t2}")
                    nc.vector.reciprocal_approx_fast(
                        out=rrow[:], in_=drow[:])
                    g = gp.tile([64, 512], f32, tag="g", name=f"g{hl}_{t2}")
                    nc.gpsimd.partition_broadcast(g[:], rrow[:], channels=64)
                    c0 = t2 % 2
                    q0 = t2 // 2
                    for nn in range(2):
                        # o * (1/denom) -> outun[c][band, q0-major cols]
                        dst = outun[hl][c0][
                            nn * 64 : nn * 64 + 64,
                            q0 * 256 : (q0 + 1) * 256]
                        nc.vector.tensor_mul(
                            dst,
                            ops[0:64, nn * 256 : (nn + 1) * 256],
                            g[0:64, nn * 256 : (nn + 1) * 256])

                if t2 % 2 == 0:
                    continue
                # ---- quarter q0 = t2//2 complete: projection ----
                q0 = t2 // 2
                for hl in range(2):
                    # proj fc in {2q0, 2q0+1}; rows hl*1024+half*512+q0+4r
                    ps2 = psc.tile([128, 512], f32, tag="cnv",
                                   name=f"psproj{hl}_{q0}")
                    for half in range(2):
                        fc = 2 * q0 + half
                        for c0 in range(2):
                            nc.tensor.matmul(
                                ps2[:, half * 256 : (half + 1) * 256],
                                lhsT=outun[hl][c0][
                                    :, fc * 128 : (fc + 1) * 128],
                                rhs=wp_sb[c0][:],
                                start=(c0 == 0), stop=(c0 == 1))
                    osb = outp.tile([128, 512], f32, tag="osb",
                                    name=f"osb{hl}_{q0}")
                    nc.vector.tensor_add(osb[:], ps2[:], bp_bc[:])
                    dstap = bass.AP(
                        tensor=out,
                        offset=(hl * 1024 + q0) * 256,
                        ap=[[4 * 256, 128], [512 * 256, 2], [1, 256]])
                    nc.sync.dma_start(
                        dstap,
                        osb[:].rearrange("p (h c) -> p h c", h=2))

    nc.compile()
    return nc


def _prep_inputs(x, y, q_w, q_gamma, q_beta, q_mean, q_var,
                 kv_w, kv_gamma, kv_beta, kv_mean, kv_var, proj_w, proj_b):
    f = np.float32
    bf = ml_dtypes.bfloat16
    x = np.ascontiguousarray(np.asarray(x, f))
    y = np.ascontiguousarray(np.asarray(y, f))

    gq = np.asarray(q_gamma, f) / np.sqrt(np.asarray(q_var, f) + BN_EPS)
    bq_full = np.asarray(q_beta, f) - np.asarray(q_mean, f) * gq
    wq_host = np.ascontiguousarray((np.asarray(q_w, f) * gq[:, None]).T).astype(bf)

    gkv = np.asarray(kv_gamma, f) / np.sqrt(np.asarray(kv_var, f) + BN_EPS)
    bkv_full = np.asarray(kv_beta, f) - np.asarray(kv_mean, f) * gkv
    wkv_host = np.ascontiguousarray((np.asarray(kv_w, f) * gkv[:, None]).T).astype(bf)

    wp_host = np.ascontiguousarray(np.asarray(proj_w, f).T).astype(bf)
    bp_host = np.asarray(proj_b, f)

    bq2 = np.tile(bq_full[None, :], (1, 2)).astype(bf)
    bkv4_h = np.tile(bkv_full[None, :], (1, 4)).astype(bf)
    bkvr_h = bkv_full[None, :].astype(bf)
    bp2 = np.tile(bp_host[None, :], (1, 2)).astype(f)

    in_maps = []
    for core in range(8):
        b, hp = core // 2, core % 2
        X4 = x[b].reshape(C, N_TOK)
        Y4 = y[b].reshape(C, N_TOK)
        xqa = np.ascontiguousarray(
            X4.reshape(C, 16, 256)[:, :, hp * 128 : (hp + 1) * 128]
        ).reshape(C, 2048).astype(bf)
        Y8 = Y4.reshape(C, 8, 512)
        yka = np.ascontiguousarray(
            Y8[:, :, hp * 128 : (hp + 1) * 128]).reshape(C, 1024).astype(bf)
        yva = np.ascontiguousarray(
            Y8[:, :, 256 + hp * 128 : 256 + (hp + 1) * 128]
        ).reshape(C, 1024).astype(bf)
        in_maps.append({
            "xq": xqa, "yk": yka, "yv": yva,
            "wq": wq_host, "bq": bq2,
            "wkv": wkv_host, "bkv4": bkv4_h, "bkvr": bkvr_h,
            "wp": wp_host, "bp": bp2,
        })
    return in_maps


def _get_nc():
    if "nc" not in _CACHE:
        _CACHE["nc"] = _build()
    return _CACHE["nc"]


def kernel(x, y, H=64, W=64, q_w=None, q_gamma=None, q_beta=None, q_mean=None,
           q_var=None, kv_w=None, kv_gamma=None, kv_beta=None, kv_mean=None,
           kv_var=None, proj_w=None, proj_b=None, _trace=False):
    from concourse.bass_utils import run_bass_kernel_spmd

    nc = _get_nc()
    in_maps = _prep_inputs(x, y, q_w, q_gamma, q_beta, q_mean, q_var,
                           kv_w, kv_gamma, kv_beta, kv_mean, kv_var,
                           proj_w, proj_b)
    kw = {}
    if _trace:
        kw = {"trace": True, "trace_cores": list(range(8))}
    res = run_bass_kernel_spmd(nc, in_maps, list(range(8)), **kw)
    outa = np.empty((B, N_TOK, C), np.float32)
    for core in range(8):
        b, hp = core // 2, core % 2
        outa[b, hp * 2048 : (hp + 1) * 2048, :] = res.results[core]["out"]
    if _trace:
        return outa, res
    return outa
